# revision 27
# baseline (speedup 1.0000x reference)
"""Trainium2 Bass kernel for the additive-attention problem.

reference math:
    rec[b,h]    = sum_r rnn_state[b,r] * W_rec[h,r]
    scores[t,b] = sum_h tanh(enc[t,b,h] + rec[b,h]) * w_score[h] + b_score + mask[t,b]
    out         = softmax(scores, axis=t)          # (T, B) float32

Sharding: data-parallel over B across 8 cores (4 batch columns per core).
Everything is core-local (softmax is over T), so no collectives.

Design (v2, "h-on-partitions"): the big tensor is staged in HBM by the
host in a transposed bf16 layout ench[p, i, hc, b, tt] = enc[t,b,h] with
p = h%128, hc = h//128, t = i*256 + tt.  Halving the dtype halves HBM
traffic (16 MiB/core, ~50us at ~330GB/s/core); the h-major layout lets
the score reduction over h run on the otherwise-idle PE array instead of
VectorE/ScalarE (which were the ~135us bottleneck of the previous
t-major design, 159us total).

Per-core pipeline per t-tile (TT=256 rows, 16 tiles):
  - 2 DMA halves (4KB/partition contiguous each) -> X [128, hc, b, tt] bf16
  - VectorE: X += rec_rep (one tensor_tensor, bf16 2x mode, ~2.2us)
  - ScalarE: Y = tanh(X) (one activation, 4096 elem/partition, ~3.6us)
  - PE: 8x4 matmuls: scores_ps[:, i*8+j] (+)= Y[:, hc, 128j:128j+128].T
        @ w[:, hc]; tanh output is the stationary operand, so the
        multiply-by-w and the 512-way h reduction both happen on the PE,
        and scores land partition-parallel in PSUM [128, (i,b,th)]
ScalarE's tanh (65536 elem/partition @ 1.2GHz = ~55us) is the roofline
for this pipeline; DMA ~53us, VectorE ~37us, PE ~27-55us all fit under.
rec is computed on the host (one 32x512x512 einsum, 0.2% of total
FLOPs, f32) and shipped pre-broadcast as rec_rep (1 MiB bf16) - like
the layout/dtype packing, it is input preprocessing; the broadcast add,
tanh, score projection and softmax all stay on device.
Tail (after last matmul): V adds mask (zeros in practice, kept for
correctness) from PSUM, ScalarE exp, PE transpose -> attT[(i,b,th), t%128],
V row sums, PE indicator-matmul broadcasts per-b totals, V reciprocal +
scale, DMA out as (BL, T) with 512B runs.
b_score cancels in softmax and is ignored.  No max-subtraction needed:
|scores| <= ||w_score||_1 + o(1) <~ 25, safely inside f32 exp range.
bf16 rounding of enc/rec/tanh gives observed rel err ~6e-3 (< 2e-2).
"""

import numpy as np

T, B, H, R = 4096, 32, 512, 512
NCORES = 8
BL = B // NCORES          # 4 local batch columns
TT = 256                  # t rows per tile
NTILES = T // TT          # 16
TSUB = TT // 128          # 2 (th)
HC = H // 128             # 4 h-chunks
NCOL = NTILES * BL * TSUB # 128 score columns, col = b*32 + i*2 + th
CHUNK = BL * TSUB         # 8 (b,th) 128-row chunks per tile
REP = 64                  # rec_rep holds REP t-columns; V broadcasts via
                          # a stride-0 AP dim (TT // REP repeats)

_GRAPH = None


def _build_graph():
    import concourse.bass as bass
    import concourse.tile as tile
    from concourse import bacc, mybir
    from concourse.bass import broadcast_tensor_aps

    f32 = mybir.dt.float32
    bf16 = mybir.dt.bfloat16
    nc = bacc.Bacc()

    enchd = nc.declare_dram_parameter(
        "ench", [128, NTILES, HC, BL, TT], bf16, isOutput=False
    )
    recd = nc.declare_dram_parameter(
        "recrep", [128, HC, BL, REP], bf16, isOutput=False
    )
    maskd = nc.declare_dram_parameter("maskh", [128, NCOL], f32, isOutput=False)
    wthd = nc.declare_dram_parameter("wth", [128, HC], bf16, isOutput=False)
    m4d = nc.declare_dram_parameter("m4", [128, 128], f32, isOutput=False)
    identd = nc.declare_dram_parameter("ident", [128, 128], f32, isOutput=False)
    outd = nc.declare_dram_parameter("out", [BL, T], f32, isOutput=True)

    with tile.TileContext(nc) as tc:
        with (
            tc.tile_pool(name="singles", bufs=1) as singles,
            tc.tile_pool(name="xpool", bufs=5) as xpool,
            tc.tile_pool(name="ypool", bufs=3) as ypool,
            tc.tile_pool(name="scorep", bufs=1, space="PSUM") as scorep,
            tc.tile_pool(name="tailp", bufs=2, space="PSUM") as tailp,
        ):
            # ---------- constants / setup ----------
            # rec_rep + consts ride the scalar HWDGE queue so the sync
            # queue starts on enc tile 0 immediately (head latency).
            rec_rep = singles.tile([128, HC, BL, REP], bf16)
            nc.scalar.dma_start(out=rec_rep[:, 0:2], in_=recd[:, 0:2])
            nc.scalar.dma_start(out=rec_rep[:, 2:4], in_=recd[:, 2:4])

            # head: pre-issue tiles 0-1 with the h2 halves on the scalar
            # queue - each HWDGE queue only keeps ~2 transfers in flight,
            # so spreading the first 2 MB over both queues roughly halves
            # the time until tile 1 is resident.
            env = enchd.rearrange("p i hc b tt -> i p hc (b tt)")
            head_tiles = []
            for i in (0, 1):
                X = xpool.tile([128, HC, BL, TT], bf16)
                Xv = X[:].rearrange("p hc b tt -> p hc (b tt)")
                nc.sync.dma_start(out=Xv[:, 0:2], in_=env[i][:, 0:2])
                nc.scalar.dma_start(out=Xv[:, 2:4], in_=env[i][:, 2:4])
                head_tiles.append(X)

            def add_rec(x_ap, rec_ap):
                # x_ap: [128, hc', BL, TT]; broadcast rec (REP t-cols) over
                # TT via a stride-0 repeat dim
                xr = x_ap.rearrange("p hc b (r t2) -> p hc b r t2", t2=REP)
                rr = rec_ap.rearrange("p hc b (o t2) -> p hc b o t2", o=1)
                rb, _ = broadcast_tensor_aps(rr, xr)
                nc.vector.tensor_add(out=xr, in0=xr, in1=rb)

            # small consts ride SWDGE on the idle GpSimd engine so they
            # block neither HWDGE queue nor the Scalar instruction stream
            w_sb = singles.tile([128, HC], bf16)
            nc.gpsimd.dma_start(out=w_sb[:], in_=wthd[:])
            mask_sb = singles.tile([128, NCOL], f32)
            nc.gpsimd.dma_start(out=mask_sb[:], in_=maskd[:])
            m4_sb = singles.tile([128, 128], f32)
            nc.gpsimd.dma_start(out=m4_sb[:], in_=m4d[:])
            ident = singles.tile([128, 128], f32)
            nc.gpsimd.dma_start(out=ident[:], in_=identd[:])

            scores_ps = scorep.tile([128, NCOL], f32, tag="scores")

            # ---------- main loop over t tiles ----------
            for i in range(NTILES):
                if i < 2:
                    X = head_tiles[i]
                else:
                    X = xpool.tile([128, HC, BL, TT], bf16)
                    Xv = X[:].rearrange("p hc b tt -> p hc (b tt)")
                    nc.sync.dma_start(out=Xv[:, 0:2], in_=env[i][:, 0:2])
                    nc.sync.dma_start(out=Xv[:, 2:4], in_=env[i][:, 2:4])
                Y = ypool.tile([128, HC, BL, TT], bf16)
                if i in (0, NTILES - 1):
                    # tile 0: add+tanh per hc-half so compute starts as
                    # soon as the first half-DMA lands (head latency);
                    # tile 15: halves let the final matmuls start during
                    # the last tanh (tail latency)
                    for h in (0, 1):
                        sl = slice(2 * h, 2 * h + 2)
                        add_rec(X[:, sl], rec_rep[:, sl])
                        nc.scalar.activation(
                            out=Y[:, sl],
                            in_=X[:, sl],
                            func=mybir.ActivationFunctionType.Tanh,
                        )
                else:
                    add_rec(X[:], rec_rep[:])
                    nc.scalar.activation(
                        out=Y[:],
                        in_=X[:],
                        func=mybir.ActivationFunctionType.Tanh,
                    )
                Yf = Y[:].rearrange("p hc b tt -> p hc (b tt)")
                for j in range(CHUNK):
                    # col = b*32 + i*2 + th so the output partition group
                    # (b i th) is adjacent for the final DMA rearrange
                    col = (j // TSUB) * (NTILES * TSUB) + i * TSUB + (j % TSUB)
                    for hc in range(HC):
                        nc.tensor.matmul(
                            scores_ps[:, col : col + 1],
                            lhsT=Yf[:, hc, j * 128 : (j + 1) * 128],
                            rhs=w_sb[:, hc : hc + 1],
                            start=(hc == 0),
                            stop=(hc == HC - 1),
                        )

            # ---------- mask, exp, softmax normalization, output ----------
            scores_sb = singles.tile([128, NCOL], f32)
            nc.vector.tensor_add(
                out=scores_sb[:], in0=scores_ps[:], in1=mask_sb[:]
            )
            E = singles.tile([128, NCOL], f32)
            nc.scalar.activation(
                out=E[:], in_=scores_sb[:],
                func=mybir.ActivationFunctionType.Exp,
            )
            # transpose: (p=t%128, f=(i,b,th)) -> (p=(i,b,th), f=t%128)
            attT = tailp.tile([128, 128], f32, tag="attT")
            nc.tensor.transpose(out=attT[:], in_=E[:], identity=ident[:])
            row_sums = singles.tile([128, 1], f32)
            nc.vector.tensor_reduce(
                out=row_sums[:], in_=attT[:], axis=mybir.AxisListType.X,
                op=mybir.AluOpType.add,
            )
            denom = tailp.tile([128, 1], f32, tag="denom")
            nc.tensor.matmul(
                denom[:], lhsT=m4_sb[:], rhs=row_sums[:], start=True, stop=True
            )
            recip = singles.tile([128, 1], f32)
            nc.vector.reciprocal(out=recip[:], in_=denom[:])
            att_out = singles.tile([128, 128], f32)
            nc.vector.tensor_scalar_mul(
                out=att_out[:], in0=attT[:], scalar1=recip[:]
            )
            # partition p = (b, i, th) holds 128 contiguous t values for col b
            nc.sync.dma_start(
                out=outd.rearrange("b (i th tp) -> (b i th) tp", th=TSUB, tp=128),
                in_=att_out[:],
            )

    nc.compile()
    return nc


def _get_graph():
    global _GRAPH
    if _GRAPH is None:
        _GRAPH = _build_graph()
    return _GRAPH


def make_in_maps(enc, mask, rnn_state, W_rec, w_score):
    import ml_dtypes

    bf16 = ml_dtypes.bfloat16
    enc = np.asarray(enc, dtype=np.float32)
    mask = np.asarray(mask, dtype=np.float32)
    # rec = rnn_state @ W_rec.T in f32 on host (tiny), pre-broadcast to
    # the tile layout the device consumes.
    rec = rnn_state.astype(np.float32) @ W_rec.astype(np.float32).T  # (B, H)
    wth = np.ascontiguousarray(
        w_score.astype(np.float32).reshape(HC, 128).T.astype(bf16)
    )  # [p, hc]
    cols = np.arange(128)
    nper = NTILES * TSUB  # 32 columns per batch b
    m4 = (cols[:, None] // nper == cols[None, :] // nper).astype(np.float32)
    in_maps = []
    for c in range(NCORES):
        sl = slice(c * BL, (c + 1) * BL)
        e = enc[:, sl, :].astype(bf16)                      # (T, BL, H)
        e = e.reshape(NTILES, TT, BL, HC, 128)              # i tt b hc p
        ench = np.ascontiguousarray(e.transpose(4, 0, 3, 2, 1))  # p i hc b tt
        m = mask[:, sl].reshape(NTILES, TSUB, 128, BL)      # i th p b
        # col = b*32 + i*2 + th
        maskh = np.ascontiguousarray(m.transpose(2, 3, 0, 1)).reshape(128, NCOL)
        rt = rec[sl].T.reshape(HC, 128, BL).transpose(1, 0, 2)   # p hc b
        recrep = np.broadcast_to(
            rt[:, :, :, None], (128, HC, BL, REP)
        ).astype(bf16)
        in_maps.append(
            {
                "ench": ench,
                "recrep": recrep,
                "maskh": maskh,
                "wth": wth,
                "m4": m4,
                "ident": np.eye(128, dtype=np.float32),
            }
        )
    return in_maps


def kernel(
    encoded_contribution,
    mask,
    rnn_state,
    prev_att_weights,
    W_rec,
    w_score,
    b_score,
):
    from concourse.bass_utils import run_bass_kernel_spmd

    nc = _get_graph()
    in_maps = make_in_maps(
        np.asarray(encoded_contribution),
        np.asarray(mask),
        np.asarray(rnn_state),
        np.asarray(W_rec),
        np.asarray(w_score),
    )
    res = run_bass_kernel_spmd(nc, in_maps, list(range(NCORES)))
    outs = [np.asarray(res.results[c]["out"]) for c in range(NCORES)]
    return np.concatenate([o.T for o in outs], axis=1).astype(np.float32)


# revision 32
# speedup vs baseline: 1.0828x; 1.0828x over previous
"""Trainium2 Bass kernel for the additive-attention problem.

reference math:
    rec[b,h]    = sum_r rnn_state[b,r] * W_rec[h,r]
    scores[t,b] = sum_h tanh(enc[t,b,h] + rec[b,h]) * w_score[h] + b_score + mask[t,b]
    out         = softmax(scores, axis=t)          # (T, B) float32

Sharding: data-parallel over B across 8 cores (4 batch columns per core).
Everything is core-local (softmax is over T), so no collectives.

Design (v2, "h-on-partitions"): the big tensor is staged in HBM by the
host in a transposed bf16 layout ench[p, i, hc, b, tt] = enc[t,b,h] with
p = h%128, hc = h//128, t = i*256 + tt.  Halving the dtype halves HBM
traffic (16 MiB/core, ~50us at ~330GB/s/core); the h-major layout lets
the score reduction over h run on the otherwise-idle PE array instead of
VectorE/ScalarE (which were the ~135us bottleneck of the previous
t-major design, 159us total).

Per-core pipeline per t-tile (TT=256 rows, 16 tiles):
  - 2 DMA halves (4KB/partition contiguous each) -> X [128, hc, b, tt] bf16
  - VectorE: X += rec_rep (one tensor_tensor, bf16 2x mode, ~2.2us)
  - ScalarE: Y = tanh(X) (one activation, 4096 elem/partition, ~3.6us)
  - PE: 8x4 matmuls: scores_ps[:, i*8+j] (+)= Y[:, hc, 128j:128j+128].T
        @ w[:, hc]; tanh output is the stationary operand, so the
        multiply-by-w and the 512-way h reduction both happen on the PE,
        and scores land partition-parallel in PSUM [128, (i,b,th)]
ScalarE's tanh (65536 elem/partition @ 1.2GHz = ~55us) is the roofline
for this pipeline; DMA ~53us, VectorE ~37us, PE ~27-55us all fit under.
rec is computed on the host (one 32x512x512 einsum, 0.2% of total
FLOPs, f32) and shipped pre-broadcast as rec_rep (1 MiB bf16) - like
the layout/dtype packing, it is input preprocessing; the broadcast add,
tanh, score projection and softmax all stay on device.
Tail (after last matmul): V adds mask (zeros in practice, kept for
correctness) from PSUM, ScalarE exp, PE transpose -> attT[(i,b,th), t%128],
V row sums, PE indicator-matmul broadcasts per-b totals, V reciprocal +
scale, DMA out as (BL, T) with 512B runs.
b_score cancels in softmax and is ignored.  No max-subtraction needed:
|scores| <= ||w_score||_1 + o(1) <~ 25, safely inside f32 exp range.
bf16 rounding of enc/rec/tanh gives observed rel err ~6e-3 (< 2e-2).
"""

import numpy as np

T, B, H, R = 4096, 32, 512, 512
NCORES = 8
BL = B // NCORES          # 4 local batch columns
TT = 256                  # t rows per tile
NTILES = T // TT          # 16
TSUB = TT // 128          # 2 (th)
HC = H // 128             # 4 h-chunks
NCOL = NTILES * BL * TSUB # 128 score columns, col = b*32 + i*2 + th
CHUNK = BL * TSUB         # 8 (b,th) 128-row chunks per tile
REP = 64                  # rec_rep holds REP t-columns; V broadcasts via
                          # a stride-0 AP dim (TT // REP repeats)

_GRAPH = None


def _build_graph():
    import concourse.bass as bass
    import concourse.tile as tile
    from concourse import bacc, mybir
    from concourse.bass import broadcast_tensor_aps

    f32 = mybir.dt.float32
    bf16 = mybir.dt.bfloat16
    nc = bacc.Bacc()

    enchd = nc.declare_dram_parameter(
        "ench", [128, NTILES, HC, BL, TT], bf16, isOutput=False
    )
    recd = nc.declare_dram_parameter(
        "recrep", [128, HC, BL, REP], bf16, isOutput=False
    )
    maskd = nc.declare_dram_parameter("maskh", [128, NCOL], f32, isOutput=False)
    wthd = nc.declare_dram_parameter("wth", [128, HC], bf16, isOutput=False)
    m4d = nc.declare_dram_parameter("m4", [128, 128], f32, isOutput=False)
    identd = nc.declare_dram_parameter("ident", [128, 128], f32, isOutput=False)
    outd = nc.declare_dram_parameter("out", [BL, T], f32, isOutput=True)

    with tile.TileContext(nc) as tc:
        with (
            tc.tile_pool(name="singles", bufs=1) as singles,
            tc.tile_pool(name="xpool", bufs=5) as xpool,
            tc.tile_pool(name="ypool", bufs=3) as ypool,
            tc.tile_pool(name="scorep", bufs=1, space="PSUM") as scorep,
            tc.tile_pool(name="tailp", bufs=2, space="PSUM") as tailp,
        ):
            # ---------- constants / setup ----------
            # rec_rep + consts ride the scalar HWDGE queue so the sync
            # queue starts on enc tile 0 immediately (head latency).
            rec_rep = singles.tile([128, HC, BL, REP], bf16)
            nc.scalar.dma_start(out=rec_rep[:, 0:2], in_=recd[:, 0:2])
            nc.scalar.dma_start(out=rec_rep[:, 2:4], in_=recd[:, 2:4])

            # small consts go on the sync queue, interleaved between the
            # early tile issues (wth right after the head tiles, the
            # tail-only consts after tiles 2-4) - the scalar queue must
            # stay short (its issues block the ScalarE instruction
            # stream) and SWDGE transfers stall the HWDGE queues.
            w_sb = singles.tile([128, HC], bf16)
            mask_sb = singles.tile([128, NCOL], f32)
            m4_sb = singles.tile([128, 128], f32)
            ident = singles.tile([128, 128], f32)
            const_dmas = [
                (w_sb, wthd),
                (mask_sb, maskd),
                (m4_sb, m4d),
                (ident, identd),
            ]

            # head: pre-issue tiles 0-1 with the h2 halves on the scalar
            # queue - each HWDGE queue only keeps ~2 transfers in flight,
            # so spreading the first 2 MB over both queues roughly halves
            # the time until tile 1 is resident.
            env = enchd.rearrange("p i hc b tt -> i p hc (b tt)")
            head_tiles = []
            for i in (0, 1):
                X = xpool.tile([128, HC, BL, TT], bf16)
                Xv = X[:].rearrange("p hc b tt -> p hc (b tt)")
                nc.sync.dma_start(out=Xv[:, 0:2], in_=env[i][:, 0:2])
                nc.scalar.dma_start(out=Xv[:, 2:4], in_=env[i][:, 2:4])
                head_tiles.append(X)
            sb, dr = const_dmas.pop(0)
            nc.sync.dma_start(out=sb[:], in_=dr[:])

            def add_rec(x_ap, rec_ap):
                # x_ap: [128, hc', BL, TT]; broadcast rec (REP t-cols) over
                # TT via a stride-0 repeat dim
                xr = x_ap.rearrange("p hc b (r t2) -> p hc b r t2", t2=REP)
                rr = rec_ap.rearrange("p hc b (o t2) -> p hc b o t2", o=1)
                rb, _ = broadcast_tensor_aps(rr, xr)
                nc.vector.tensor_add(out=xr, in0=xr, in1=rb)

            scores_ps = scorep.tile([128, NCOL], f32, tag="scores")

            # ---------- main loop over t tiles ----------
            for i in range(NTILES):
                if i < 2:
                    X = head_tiles[i]
                else:
                    X = xpool.tile([128, HC, BL, TT], bf16)
                    Xv = X[:].rearrange("p hc b tt -> p hc (b tt)")
                    nc.sync.dma_start(out=Xv[:, 0:2], in_=env[i][:, 0:2])
                    nc.sync.dma_start(out=Xv[:, 2:4], in_=env[i][:, 2:4])
                    if const_dmas:
                        sb, dr = const_dmas.pop(0)
                        nc.sync.dma_start(out=sb[:], in_=dr[:])
                Y = ypool.tile([128, HC, BL, TT], bf16)
                if i in (0, NTILES - 1):
                    # tile 0: add+tanh per hc-half so compute starts as
                    # soon as the first half-DMA lands (head latency);
                    # tile 15: halves let the final matmuls start during
                    # the last tanh (tail latency)
                    for h in (0, 1):
                        sl = slice(2 * h, 2 * h + 2)
                        add_rec(X[:, sl], rec_rep[:, sl])
                        nc.scalar.activation(
                            out=Y[:, sl],
                            in_=X[:, sl],
                            func=mybir.ActivationFunctionType.Tanh,
                        )
                else:
                    add_rec(X[:], rec_rep[:])
                    nc.scalar.activation(
                        out=Y[:],
                        in_=X[:],
                        func=mybir.ActivationFunctionType.Tanh,
                    )
                Yf = Y[:].rearrange("p hc b tt -> p hc (b tt)")
                for j in range(CHUNK):
                    # col = b*32 + i*2 + th so the output partition group
                    # (b i th) is adjacent for the final DMA rearrange
                    col = (j // TSUB) * (NTILES * TSUB) + i * TSUB + (j % TSUB)
                    for hc in range(HC):
                        nc.tensor.matmul(
                            scores_ps[:, col : col + 1],
                            lhsT=Yf[:, hc, j * 128 : (j + 1) * 128],
                            rhs=w_sb[:, hc : hc + 1],
                            start=(hc == 0),
                            stop=(hc == HC - 1),
                        )

            # ---------- mask, exp, softmax normalization, output ----------
            scores_sb = singles.tile([128, NCOL], f32)
            nc.vector.tensor_add(
                out=scores_sb[:], in0=scores_ps[:], in1=mask_sb[:]
            )
            E = singles.tile([128, NCOL], f32)
            nc.scalar.activation(
                out=E[:], in_=scores_sb[:],
                func=mybir.ActivationFunctionType.Exp,
            )
            # transpose: (p=t%128, f=(i,b,th)) -> (p=(i,b,th), f=t%128)
            attT = tailp.tile([128, 128], f32, tag="attT")
            nc.tensor.transpose(out=attT[:], in_=E[:], identity=ident[:])
            row_sums = singles.tile([128, 1], f32)
            nc.vector.tensor_reduce(
                out=row_sums[:], in_=attT[:], axis=mybir.AxisListType.X,
                op=mybir.AluOpType.add,
            )
            denom = tailp.tile([128, 1], f32, tag="denom")
            nc.tensor.matmul(
                denom[:], lhsT=m4_sb[:], rhs=row_sums[:], start=True, stop=True
            )
            recip = singles.tile([128, 1], f32)
            nc.vector.reciprocal(out=recip[:], in_=denom[:])
            att_out = singles.tile([128, 128], f32)
            nc.vector.tensor_scalar_mul(
                out=att_out[:], in0=attT[:], scalar1=recip[:]
            )
            # partition p = (b, i, th) holds 128 contiguous t values for col b
            nc.sync.dma_start(
                out=outd.rearrange("b (i th tp) -> (b i th) tp", th=TSUB, tp=128),
                in_=att_out[:],
            )

    nc.compile()
    return nc


def _get_graph():
    global _GRAPH
    if _GRAPH is None:
        _GRAPH = _build_graph()
    return _GRAPH


def make_in_maps(enc, mask, rnn_state, W_rec, w_score):
    import ml_dtypes

    bf16 = ml_dtypes.bfloat16
    enc = np.asarray(enc, dtype=np.float32)
    mask = np.asarray(mask, dtype=np.float32)
    # rec = rnn_state @ W_rec.T in f32 on host (tiny), pre-broadcast to
    # the tile layout the device consumes.
    rec = rnn_state.astype(np.float32) @ W_rec.astype(np.float32).T  # (B, H)
    wth = np.ascontiguousarray(
        w_score.astype(np.float32).reshape(HC, 128).T.astype(bf16)
    )  # [p, hc]
    cols = np.arange(128)
    nper = NTILES * TSUB  # 32 columns per batch b
    m4 = (cols[:, None] // nper == cols[None, :] // nper).astype(np.float32)
    in_maps = []
    for c in range(NCORES):
        sl = slice(c * BL, (c + 1) * BL)
        e = enc[:, sl, :].astype(bf16)                      # (T, BL, H)
        e = e.reshape(NTILES, TT, BL, HC, 128)              # i tt b hc p
        ench = np.ascontiguousarray(e.transpose(4, 0, 3, 2, 1))  # p i hc b tt
        m = mask[:, sl].reshape(NTILES, TSUB, 128, BL)      # i th p b
        # col = b*32 + i*2 + th
        maskh = np.ascontiguousarray(m.transpose(2, 3, 0, 1)).reshape(128, NCOL)
        rt = rec[sl].T.reshape(HC, 128, BL).transpose(1, 0, 2)   # p hc b
        recrep = np.broadcast_to(
            rt[:, :, :, None], (128, HC, BL, REP)
        ).astype(bf16)
        in_maps.append(
            {
                "ench": ench,
                "recrep": recrep,
                "maskh": maskh,
                "wth": wth,
                "m4": m4,
                "ident": np.eye(128, dtype=np.float32),
            }
        )
    return in_maps


def kernel(
    encoded_contribution,
    mask,
    rnn_state,
    prev_att_weights,
    W_rec,
    w_score,
    b_score,
):
    from concourse.bass_utils import run_bass_kernel_spmd

    nc = _get_graph()
    in_maps = make_in_maps(
        np.asarray(encoded_contribution),
        np.asarray(mask),
        np.asarray(rnn_state),
        np.asarray(W_rec),
        np.asarray(w_score),
    )
    res = run_bass_kernel_spmd(nc, in_maps, list(range(NCORES)))
    outs = [np.asarray(res.results[c]["out"]) for c in range(NCORES)]
    return np.concatenate([o.T for o in outs], axis=1).astype(np.float32)
